# revision 7
# baseline (speedup 1.0000x reference)
"""4-D average pool (kernel=2, stride=2) over [2,16,32,32,32,32] f32, on 8 NeuronCores.

Strategy: data-parallel over the 32 (b,c) slices -> 4 slices per core.  The
host folds the 1/16 scale into a bf16 cast (tolerance is 2e-2; measured
error ~8e-3), halving the HBM stream to 8 MiB/core, and permutes the shard
so each SBUF partition receives a complete 4x4 pooling group:

  rows (d1,d2) -> (a=d1/2, c2=d2/2, e2=d2%2, e1=d1%2): partition p of a
    512-row load holds the 4 rows of output group (a,c2)
  cols (d3,d4) -> (e4=d4%2, d3, o4=d4/2): d4 partners sit in separate
    512-col planes

With that layout the whole reduction is FOUR contiguous bf16 DVE adds per
load (pool d3, then e1, e2, e4 -> all 2x mode), no matmul, no PSUM, no
copies.  Loads go p-major (contiguous HBM runs per partition) on both HWDGE
rings (SP via nc.sync, ACT via nc.scalar), all triggered up front, ~430 GB/s
combined.  Ring bytes are UNEVEN (sync 4.5 MiB, scalar 3.5 MiB) and the last
two 1 MiB row-blocks are split into 512 KiB column halves: the final chunk
to land is a half whose partner arrived earlier, so the post-stream drain is
one half-chain (~1.3 us) instead of two full chains (~4.7 us).  Stores are
bf16 [128, 256] per block; the host upcasts to f32 and decodes y
[128, 2048] back to (B,C,16,16,16,16).
"""

import sys

import ml_dtypes
import numpy as np

if "/opt/trn_rl_repo" not in sys.path:
    sys.path.insert(0, "/opt/trn_rl_repo")

import concourse.bacc as bacc
import concourse.bass as bass
import concourse.tile as tile
from concourse import mybir
from concourse.bass_utils import run_bass_kernel_spmd

N_CORES = 8
SLICES_PER_CORE = 4  # 32 (b,c) slices / 8 cores
ROWS = SLICES_PER_CORE * 1024  # 4096
N_BLOCKS = 8  # 512-row output blocks; last two split into column halves
BF16 = mybir.dt.bfloat16


def build_nc() -> bass.Bass:
    nc = bacc.Bacc()
    x = nc.dram_tensor("x", [ROWS, 1024], BF16, kind="ExternalInput")
    y = nc.dram_tensor("y", [128, 256 * N_BLOCKS], BF16, kind="ExternalOutput")

    with tile.TileContext(nc) as tc:
        with (
            # whole 8 MiB shard SBUF-resident: no slot reuse, loads carry no
            # waits and stream back-to-back
            tc.tile_pool(name="inp", bufs=6) as inp,
            tc.tile_pool(name="inh", bufs=4) as inh,
            tc.tile_pool(name="m1p", bufs=3) as m1p,
            tc.tile_pool(name="m2p", bufs=3) as m2p,
            tc.tile_pool(name="m3p", bufs=3) as m3p,
            tc.tile_pool(name="m1h", bufs=4) as m1hp,
            tc.tile_pool(name="m2h", bufs=4) as m2hp,
            tc.tile_pool(name="m3h", bufs=4) as m3hp,
            tc.tile_pool(name="obp", bufs=4) as obp,
        ):
            # Load triggers, all up front.  Per-ring FIFO order makes block
            # 7's second column-half the last chunk to land, alone:
            #   sync   (4.5 MiB): L0, L2, L4, L6a, L6b, L7b
            #   scalar (3.5 MiB): L1, L3, L5, L7a
            full_tiles = {}
            half_tiles = {}

            def load_full(k, ring):
                t = inp.tile([128, 4096], BF16, tag="t")
                src = x[512 * k : 512 * (k + 1), :].rearrange(
                    "(p r) c -> p r c", p=128
                )
                ring.dma_start(t[:].rearrange("p (r c) -> p r c", r=4), src)
                full_tiles[k] = t

            def load_half(k, h, ring):
                th = inh.tile([128, 2048], BF16, tag="th")
                src = x[
                    512 * k : 512 * (k + 1), 512 * h : 512 * (h + 1)
                ].rearrange("(p r) c -> p r c", p=128)
                ring.dma_start(th[:].rearrange("p (r c) -> p r c", r=4), src)
                half_tiles[(k, h)] = th

            load_full(0, nc.sync)
            load_full(1, nc.scalar)
            load_full(2, nc.sync)
            load_full(3, nc.scalar)
            load_full(4, nc.sync)
            load_full(5, nc.scalar)
            load_half(6, 0, nc.sync)
            load_half(7, 0, nc.scalar)
            load_half(6, 1, nc.sync)
            load_half(7, 1, nc.sync)

            rings = [nc.sync, nc.scalar]

            for k in range(6):
                t = full_tiles[k]
                # A: pool d3 pairs (runs of 16, g = (e2,e1,e4) collapsed)
                v = t[:].rearrange(
                    "p (g o3 e3 o4) -> p g o3 e3 o4", g=8, o3=16, o4=16
                )
                m1 = m1p.tile([128, 2048], BF16, tag="m1")
                m1v = m1[:].rearrange("p (g o3 o4) -> p g o3 o4", g=8, o3=16)
                nc.vector.tensor_add(m1v, v[:, :, :, 0, :], v[:, :, :, 1, :])

                # B: pool e1 = d1 pairs
                w = m1[:].rearrange("p (e2 e1 f) -> p e2 e1 f", e2=2, e1=2)
                m2 = m2p.tile([128, 1024], BF16, tag="m2")
                m2v = m2[:].rearrange("p (e2 f) -> p e2 f", e2=2)
                nc.vector.tensor_add(m2v, w[:, :, 0, :], w[:, :, 1, :])

                # C: pool e2 = d2 pairs
                w2 = m2[:].rearrange("p (e2 f) -> p e2 f", e2=2)
                m3 = m3p.tile([128, 512], BF16, tag="m3")
                nc.vector.tensor_add(m3[:], w2[:, 0, :], w2[:, 1, :])

                # D: pool e4 = d4 pairs
                w3 = m3[:].rearrange("p (e4 f) -> p e4 f", e4=2)
                ob = obp.tile([128, 256], BF16, tag="ob")
                nc.vector.tensor_add(ob[:], w3[:, 0, :], w3[:, 1, :])

                rings[k % 2].dma_start(y[:, 256 * k : 256 * (k + 1)], ob[:])

            for k in (6, 7):
                # column-half chains: pool d3/e1/e2 within each e4 plane,
                # then join the planes (D)
                m3h = {}
                for h in (0, 1):
                    th = half_tiles[(k, h)]
                    v = th[:].rearrange(
                        "p (g o3 e3 o4) -> p g o3 e3 o4", g=4, o3=16, o4=16
                    )
                    m1 = m1hp.tile([128, 1024], BF16, tag="m1h")
                    m1v = m1[:].rearrange(
                        "p (g o3 o4) -> p g o3 o4", g=4, o3=16
                    )
                    nc.vector.tensor_add(
                        m1v, v[:, :, :, 0, :], v[:, :, :, 1, :]
                    )
                    w = m1[:].rearrange(
                        "p (e2 e1 f) -> p e2 e1 f", e2=2, e1=2
                    )
                    m2 = m2hp.tile([128, 512], BF16, tag="m2h")
                    m2v = m2[:].rearrange("p (e2 f) -> p e2 f", e2=2)
                    nc.vector.tensor_add(m2v, w[:, :, 0, :], w[:, :, 1, :])
                    w2 = m2[:].rearrange("p (e2 f) -> p e2 f", e2=2)
                    m3 = m3hp.tile([128, 256], BF16, tag="m3h")
                    nc.vector.tensor_add(m3[:], w2[:, 0, :], w2[:, 1, :])
                    m3h[h] = m3

                ob = obp.tile([128, 256], BF16, tag="ob")
                nc.vector.tensor_add(ob[:], m3h[0][:], m3h[1][:])
                rings[k % 2].dma_start(y[:, 256 * k : 256 * (k + 1)], ob[:])

    nc.compile()
    return nc


_NC_CACHE: bass.Bass | None = None


def kernel(nd_tensor: np.ndarray, _trace: bool = False):
    global _NC_CACHE
    x = np.ascontiguousarray(np.asarray(nd_tensor, dtype=np.float32)).reshape(
        32, 1024, 1024
    )
    xb = (x * 0.0625).astype(ml_dtypes.bfloat16)  # fold the 1/16 avg scale
    # rows (a, e1, c2, e2) -> (a, c2, e2, e1); cols (d3, o4, e4) -> (e4, d3, o4)
    xb = np.ascontiguousarray(
        xb.reshape(32, 16, 2, 16, 2, 32, 16, 2).transpose(0, 1, 3, 4, 2, 7, 5, 6)
    ).reshape(32, 1024, 1024)
    if _NC_CACHE is None:
        _NC_CACHE = build_nc()
    nc = _NC_CACHE

    in_maps = [
        {
            "x": xb[SLICES_PER_CORE * i : SLICES_PER_CORE * (i + 1)].reshape(
                ROWS, 1024
            )
        }
        for i in range(N_CORES)
    ]
    res = run_bass_kernel_spmd(
        nc, in_maps, core_ids=list(range(N_CORES)), trace=_trace
    )
    # y[p, 256k + 16*o3 + o4]: k = (s_local 4, khalf 2); group index
    # q = 128*khalf + p = (a 16, c2 16) -> out[4i+s_local, a, c2, o3, o4].
    outs = []
    for i in range(N_CORES):
        yc = res.results[i]["y"].astype(np.float32)
        yc = yc.reshape(128, 4, 2, 16, 16).transpose(1, 2, 0, 3, 4)
        outs.append(yc.reshape(4, 16, 16, 16, 16))
    out = np.concatenate(outs, axis=0).reshape(2, 16, 16, 16, 16, 16)
    out = np.ascontiguousarray(out).astype(np.float32)
    if _trace:
        kernel.last_results = res
    return out


# revision 8
# speedup vs baseline: 1.0336x; 1.0336x over previous
"""4-D average pool (kernel=2, stride=2) over [2,16,32,32,32,32] f32, on 8 NeuronCores.

Strategy: data-parallel over the 32 (b,c) slices -> 4 slices per core.  The
host folds the 1/16 scale into a bf16 cast (tolerance 2e-2, measured ~8e-3),
halving the HBM stream to 8 MiB/core, and permutes columns so d4 partners
sit in separate 512-col planes (cols (d3,o4,e4) -> (e4,d3,o4)) -> every
on-device access is contiguous.

8 x 512-row blocks per core.  Blocks 0-5 and 7 keep natural row order
(partition p of a block = 4 consecutive rows = one d1, four d2): one DVE
add pools d3, then FOUR accumulating bf16 matmuls with a [128,64] 0/1
matrix pool the (d2 pair, d4 plane) combinations and the d1 partition
pairs -> PSUM [64,512]; ScalarE copies to bf16 and stores.  DVE is only
~50% loaded, TensorE ~50%.

Block 6 is row-permuted on the host so each partition holds a complete 4x4
group ((a,c2,e2,e1) order), loaded as two 512 KiB column halves that are
the LAST chunks in the sync ring: its reduction is pure DVE adds (no
matmul/PSUM/copy), so the post-stream drain is one ~1.3 us half-chain.

DMA plan (<= 8 HWDGE loads -> no DMAHW lane-reuse waits, loads are never
demoted behind stores): sync ring L0,L2,L4,L6a,L6b; scalar ring L1,L3,L5;
pm + block-7 load via SWDGE (GpSimd), landing mid-stream.  All load
triggers precede everything else on both ring sequencers.
"""

import sys

import ml_dtypes
import numpy as np

if "/opt/trn_rl_repo" not in sys.path:
    sys.path.insert(0, "/opt/trn_rl_repo")

import concourse.bacc as bacc
import concourse.bass as bass
import concourse.tile as tile
from concourse import mybir
from concourse.bass_utils import run_bass_kernel_spmd

N_CORES = 8
SLICES_PER_CORE = 4  # 32 (b,c) slices / 8 cores
ROWS = SLICES_PER_CORE * 1024  # 4096
BF16 = mybir.dt.bfloat16
F32 = mybir.dt.float32
# V3-path blocks (k) -> (slice, khalf); block 6 takes the all-DVE path
MM_BLOCKS = [0, 1, 2, 3, 4, 5, 7]


def _build_pm() -> np.ndarray:
    # pm[p, q] = 1 for q = 8*(p//16) + p%8: partitions p and p+8 hold the
    # (d1, d1+1) pair for the same d2 block (1/16 is folded on the host).
    b = np.zeros((128, 64), np.float32)
    for p in range(128):
        b[p, 8 * (p // 16) + p % 8] = 1.0
    return b.astype(ml_dtypes.bfloat16)


def build_nc() -> bass.Bass:
    nc = bacc.Bacc()
    x = nc.dram_tensor("x", [ROWS, 1024], BF16, kind="ExternalInput")
    pm = nc.dram_tensor("pm", [128, 64], BF16, kind="ExternalInput")
    y3 = nc.dram_tensor("y3", [64, 512 * 7], BF16, kind="ExternalOutput")
    y4 = nc.dram_tensor("y4", [128, 256], BF16, kind="ExternalOutput")

    with tile.TileContext(nc) as tc:
        with (
            tc.tile_pool(name="pmp", bufs=1) as pmp,
            tc.tile_pool(name="inp", bufs=7) as inp,
            tc.tile_pool(name="inh", bufs=2) as inh,
            tc.tile_pool(name="m1p", bufs=3) as m1p,
            tc.tile_pool(name="psp", bufs=7, space=bass.MemorySpace.PSUM) as psp,
            tc.tile_pool(name="ob3", bufs=3) as ob3p,
            tc.tile_pool(name="m1h", bufs=2) as m1hp,
            tc.tile_pool(name="m2h", bufs=2) as m2hp,
            tc.tile_pool(name="m3h", bufs=2) as m3hp,
            tc.tile_pool(name="ob4", bufs=1) as ob4p,
        ):
            pm_t = pmp.tile([128, 64], BF16)
            nc.gpsimd.dma_start(pm_t[:], pm[:])

            full_tiles = {}

            def load_full(k, ring):
                t = inp.tile([128, 4096], BF16, tag="t")
                src = x[512 * k : 512 * (k + 1), :].rearrange(
                    "(p r) c -> p r c", p=128
                )
                ring.dma_start(t[:].rearrange("p (r c) -> p r c", r=4), src)
                full_tiles[k] = t

            load_full(0, nc.sync)
            load_full(1, nc.scalar)
            load_full(2, nc.sync)
            load_full(3, nc.scalar)
            load_full(4, nc.sync)
            load_full(5, nc.scalar)
            half_tiles = {}
            for h in (0, 1):
                th = inh.tile([128, 2048], BF16, tag="th")
                src = x[3072:3584, 512 * h : 512 * (h + 1)].rearrange(
                    "(p r) c -> p r c", p=128
                )
                nc.sync.dma_start(
                    th[:].rearrange("p (r c) -> p r c", r=4), src
                )
                half_tiles[h] = th
            load_full(7, nc.gpsimd)

            rings = [nc.sync, nc.scalar]

            for j, k in enumerate(MM_BLOCKS):
                t = full_tiles[k]
                # A: pool d3 pairs; g = (d2-local, e4) collapsed
                v = t[:].rearrange(
                    "p (g o3 e3 o4) -> p g o3 e3 o4", g=8, o3=16, o4=16
                )
                m1 = m1p.tile([128, 2048], BF16, tag="m1")
                m1v = m1[:].rearrange("p (g o3 o4) -> p g o3 o4", g=8, o3=16)
                nc.vector.tensor_add(m1v, v[:, :, :, 0, :], v[:, :, :, 1, :])

                # d2/d4 pairs via 4 accumulating matmuls (contiguous rhs);
                # d1 partition pairs via the 0/1 pooling matrix.
                u = m1[:].rearrange(
                    "p (ro re e4 o3 o4) -> p ro re e4 o3 o4",
                    ro=2, re=2, e4=2, o3=16,
                )
                ps = psp.tile([64, 512], F32, tag="ps")
                for i, (a, c) in enumerate(
                    [(0, 0), (0, 1), (1, 0), (1, 1)]
                ):
                    nc.tensor.matmul(
                        ps[:],
                        pm_t[:],
                        u[:, :, a, c, :, :],
                        start=(i == 0),
                        stop=(i == 3),
                    )

                ob = ob3p.tile([64, 512], BF16, tag="ob")
                nc.scalar.copy(ob[:], ps[:])
                rings[k % 2].dma_start(y3[:, 512 * j : 512 * (j + 1)], ob[:])

            # Block 6 (rows 3072-3584, host row-permuted): pure-DVE chains
            # per column half, then join the e4 planes.
            m3h = {}
            for h in (0, 1):
                th = half_tiles[h]
                v = th[:].rearrange(
                    "p (g o3 e3 o4) -> p g o3 e3 o4", g=4, o3=16, o4=16
                )
                m1 = m1hp.tile([128, 1024], BF16, tag="m1h")
                m1v = m1[:].rearrange("p (g o3 o4) -> p g o3 o4", g=4, o3=16)
                nc.vector.tensor_add(m1v, v[:, :, :, 0, :], v[:, :, :, 1, :])
                w = m1[:].rearrange("p (e2 e1 f) -> p e2 e1 f", e2=2, e1=2)
                m2 = m2hp.tile([128, 512], BF16, tag="m2h")
                m2v = m2[:].rearrange("p (e2 f) -> p e2 f", e2=2)
                nc.vector.tensor_add(m2v, w[:, :, 0, :], w[:, :, 1, :])
                w2 = m2[:].rearrange("p (e2 f) -> p e2 f", e2=2)
                m3 = m3hp.tile([128, 256], BF16, tag="m3h")
                nc.vector.tensor_add(m3[:], w2[:, 0, :], w2[:, 1, :])
                m3h[h] = m3

            ob4 = ob4p.tile([128, 256], BF16)
            nc.vector.tensor_add(ob4[:], m3h[0][:], m3h[1][:])
            nc.sync.dma_start(y4[:], ob4[:])

    nc.compile()
    return nc


_NC_CACHE: bass.Bass | None = None


def kernel(nd_tensor: np.ndarray, _trace: bool = False):
    global _NC_CACHE
    x = np.ascontiguousarray(np.asarray(nd_tensor, dtype=np.float32)).reshape(
        32, 1024, 1024
    )
    xb = (x * 0.0625).astype(ml_dtypes.bfloat16)  # fold the 1/16 avg scale
    # cols (d3, o4, e4) -> (e4, d3, o4)
    xb = np.ascontiguousarray(
        xb.reshape(32, 1024, 32, 16, 2).transpose(0, 1, 4, 2, 3)
    ).reshape(32, 1024, 1024)
    pm = _build_pm()
    if _NC_CACHE is None:
        _NC_CACHE = build_nc()
    nc = _NC_CACHE

    in_maps = []
    for i in range(N_CORES):
        xc = np.ascontiguousarray(
            xb[SLICES_PER_CORE * i : SLICES_PER_CORE * (i + 1)]
        ).reshape(ROWS, 1024)
        # block 6 rows (aL, e1, c2, e2) -> (aL, c2, e2, e1)
        blk = xc[3072:3584].reshape(8, 2, 16, 2, 1024)
        xc[3072:3584] = blk.transpose(0, 2, 3, 1, 4).reshape(512, 1024)
        in_maps.append({"x": xc, "pm": pm})

    res = run_bass_kernel_spmd(
        nc, in_maps, core_ids=list(range(N_CORES)), trace=_trace
    )
    # y3[q, 512j + f]: q = (o1l' 8, d2blk 8), f = (o2l 2, o3 16, o4 16);
    # block j -> (slice, khalf): o1 = 8*khalf + o1l', o2 = 2*d2blk + o2l.
    # y4[p, 16*o3 + o4]: slice 3, o1 = p//16 (in [0,8)), o2 = p%16.
    outs = []
    for i in range(N_CORES):
        arr = (
            res.results[i]["y3"]
            .astype(np.float32)
            .reshape(8, 8, 7, 2, 16, 16)
            .transpose(2, 0, 3, 1, 4, 5)  # [j, o1l', o2l, d2blk, o3, o4]
        )
        oc = np.empty((4, 16, 16, 16, 16), np.float32)
        for j, (s, kh) in enumerate(
            [(0, 0), (0, 1), (1, 0), (1, 1), (2, 0), (2, 1), (3, 1)]
        ):
            # note o2 = 2*d2blk + o2l -> need [o1l', d2blk, o2l] order
            oc[s, 8 * kh : 8 * kh + 8] = (
                arr[j].transpose(0, 2, 1, 3, 4).reshape(8, 16, 16, 16)
            )
        oc[3, 0:8] = (
            res.results[i]["y4"].astype(np.float32).reshape(8, 16, 16, 16)
        )
        outs.append(oc)
    out = np.concatenate(outs, axis=0).reshape(2, 16, 16, 16, 16, 16)
    out = np.ascontiguousarray(out).astype(np.float32)
    if _trace:
        kernel.last_results = res
    return out


# revision 10
# speedup vs baseline: 1.1044x; 1.0684x over previous
"""4-D average pool (kernel=2, stride=2) over [2,16,32,32,32,32] f32, on 8 NeuronCores.

Strategy: data-parallel over the 32 (b,c) slices -> 4 slices per core.  The
host folds the 1/16 scale into a bf16 cast (tolerance 2e-2, measured ~8e-3),
halving the HBM stream to 8 MiB/core, and permutes columns so d4 partners
sit in separate 512-col planes (cols (d3,o4,e4) -> (e4,d3,o4)) -> every
on-device access is contiguous.

8 x 512-row blocks per core.  Blocks 0-5 and 7 keep natural row order
(partition p of a block = 4 consecutive rows = one d1, four d2): one DVE
add pools d3, then FOUR accumulating bf16 matmuls with a [128,64] 0/1
matrix pool the (d2 pair, d4 plane) combinations and the d1 partition
pairs -> PSUM [64,512]; ScalarE copies to bf16 and stores.  DVE is only
~50% loaded, TensorE ~50%.

Block 6 is row-permuted on the host so each partition holds a complete 4x4
group ((a,c2,e2,e1) order), loaded as two 512 KiB column halves that are
the LAST chunks in the sync ring: its reduction is pure DVE adds (no
matmul/PSUM/copy), so the post-stream drain is one ~1.3 us half-chain.

DMA plan: sync ring L0,L2,L4,L5,L6a,L6b (5 MiB) so block 6's halves land
last and alone; scalar ring pm,L1,L3,L7 (3 MiB) drains early.  All load
triggers are emitted under tc.high_priority() so the scheduler can never
demote them behind compute-waiting stores (the V5 failure mode), and the
two DMAHW lane-reuse waits fall on pm/L0's lanes, which complete early.
"""

import sys

import ml_dtypes
import numpy as np

if "/opt/trn_rl_repo" not in sys.path:
    sys.path.insert(0, "/opt/trn_rl_repo")

import concourse.bacc as bacc
import concourse.bass as bass
import concourse.tile as tile
from concourse import mybir
from concourse.bass_utils import run_bass_kernel_spmd

N_CORES = 8
SLICES_PER_CORE = 4  # 32 (b,c) slices / 8 cores
ROWS = SLICES_PER_CORE * 1024  # 4096
BF16 = mybir.dt.bfloat16
F32 = mybir.dt.float32
# V3-path blocks (k) -> (slice, khalf); block 6 takes the all-DVE path
MM_BLOCKS = [0, 1, 2, 3, 4, 5, 7]


def _build_pm() -> np.ndarray:
    # pm[p, q] = 1 for q = 8*(p//16) + p%8: partitions p and p+8 hold the
    # (d1, d1+1) pair for the same d2 block (1/16 is folded on the host).
    b = np.zeros((128, 64), np.float32)
    for p in range(128):
        b[p, 8 * (p // 16) + p % 8] = 1.0
    return b.astype(ml_dtypes.bfloat16)


def build_nc() -> bass.Bass:
    nc = bacc.Bacc()
    x = nc.dram_tensor("x", [ROWS, 1024], BF16, kind="ExternalInput")
    pm = nc.dram_tensor("pm", [128, 64], BF16, kind="ExternalInput")
    y3 = nc.dram_tensor("y3", [64, 512 * 7], BF16, kind="ExternalOutput")
    y4 = nc.dram_tensor("y4", [128, 256], BF16, kind="ExternalOutput")

    with tile.TileContext(nc) as tc:
        with (
            tc.tile_pool(name="pmp", bufs=1) as pmp,
            tc.tile_pool(name="inp", bufs=7) as inp,
            tc.tile_pool(name="inh", bufs=2) as inh,
            tc.tile_pool(name="m1p", bufs=3) as m1p,
            tc.tile_pool(name="psp", bufs=7, space=bass.MemorySpace.PSUM) as psp,
            tc.tile_pool(name="ob3", bufs=3) as ob3p,
            tc.tile_pool(name="m1h", bufs=2) as m1hp,
            tc.tile_pool(name="m2h", bufs=2) as m2hp,
            tc.tile_pool(name="m3h", bufs=2) as m3hp,
            tc.tile_pool(name="ob4", bufs=1) as ob4p,
        ):
            pm_t = pmp.tile([128, 64], BF16)
            full_tiles = {}

            def load_full(k, ring):
                t = inp.tile([128, 4096], BF16, tag="t")
                src = x[512 * k : 512 * (k + 1), :].rearrange(
                    "(p r) c -> p r c", p=128
                )
                ring.dma_start(t[:].rearrange("p (r c) -> p r c", r=4), src)
                full_tiles[k] = t

            # high_priority pins every load trigger ahead of stores/copies on
            # both ring sequencers; DMAHW lanes are assigned round-robin in
            # this order, so the 9th/10th loads reuse the lanes of pm / L0
            # (both complete early -> benign lane-reuse waits).
            half_tiles = {}
            with tc.high_priority():
                nc.scalar.dma_start(pm_t[:], pm[:])
                load_full(0, nc.sync)
                load_full(1, nc.scalar)
                load_full(2, nc.sync)
                load_full(3, nc.scalar)
                load_full(4, nc.sync)
                load_full(7, nc.scalar)
                load_full(5, nc.sync)
                for h in (0, 1):
                    th = inh.tile([128, 2048], BF16, tag="th")
                    src = x[3072:3584, 512 * h : 512 * (h + 1)].rearrange(
                        "(p r) c -> p r c", p=128
                    )
                    nc.sync.dma_start(
                        th[:].rearrange("p (r c) -> p r c", r=4), src
                    )
                    half_tiles[h] = th

            rings = [nc.sync, nc.scalar]

            for j, k in enumerate(MM_BLOCKS):
                t = full_tiles[k]
                # A: pool d3 pairs; g = (d2-local, e4) collapsed
                v = t[:].rearrange(
                    "p (g o3 e3 o4) -> p g o3 e3 o4", g=8, o3=16, o4=16
                )
                m1 = m1p.tile([128, 2048], BF16, tag="m1")
                m1v = m1[:].rearrange("p (g o3 o4) -> p g o3 o4", g=8, o3=16)
                nc.vector.tensor_add(m1v, v[:, :, :, 0, :], v[:, :, :, 1, :])

                # d2/d4 pairs via 4 accumulating matmuls (contiguous rhs);
                # d1 partition pairs via the 0/1 pooling matrix.
                u = m1[:].rearrange(
                    "p (ro re e4 o3 o4) -> p ro re e4 o3 o4",
                    ro=2, re=2, e4=2, o3=16,
                )
                ps = psp.tile([64, 512], F32, tag="ps")
                for i, (a, c) in enumerate(
                    [(0, 0), (0, 1), (1, 0), (1, 1)]
                ):
                    nc.tensor.matmul(
                        ps[:],
                        pm_t[:],
                        u[:, :, a, c, :, :],
                        start=(i == 0),
                        stop=(i == 3),
                    )

                ob = ob3p.tile([64, 512], BF16, tag="ob")
                nc.scalar.copy(ob[:], ps[:])
                rings[k % 2].dma_start(y3[:, 512 * j : 512 * (j + 1)], ob[:])

            # Block 6 (rows 3072-3584, host row-permuted): pure-DVE chains
            # per column half, then join the e4 planes.
            m3h = {}
            for h in (0, 1):
                th = half_tiles[h]
                v = th[:].rearrange(
                    "p (g o3 e3 o4) -> p g o3 e3 o4", g=4, o3=16, o4=16
                )
                m1 = m1hp.tile([128, 1024], BF16, tag="m1h")
                m1v = m1[:].rearrange("p (g o3 o4) -> p g o3 o4", g=4, o3=16)
                nc.vector.tensor_add(m1v, v[:, :, :, 0, :], v[:, :, :, 1, :])
                w = m1[:].rearrange("p (e2 e1 f) -> p e2 e1 f", e2=2, e1=2)
                m2 = m2hp.tile([128, 512], BF16, tag="m2h")
                m2v = m2[:].rearrange("p (e2 f) -> p e2 f", e2=2)
                nc.vector.tensor_add(m2v, w[:, :, 0, :], w[:, :, 1, :])
                w2 = m2[:].rearrange("p (e2 f) -> p e2 f", e2=2)
                m3 = m3hp.tile([128, 256], BF16, tag="m3h")
                nc.vector.tensor_add(m3[:], w2[:, 0, :], w2[:, 1, :])
                m3h[h] = m3

            ob4 = ob4p.tile([128, 256], BF16)
            nc.vector.tensor_add(ob4[:], m3h[0][:], m3h[1][:])
            nc.sync.dma_start(y4[:], ob4[:])

    nc.compile()
    return nc


_NC_CACHE: bass.Bass | None = None


def kernel(nd_tensor: np.ndarray, _trace: bool = False):
    global _NC_CACHE
    x = np.ascontiguousarray(np.asarray(nd_tensor, dtype=np.float32)).reshape(
        32, 1024, 1024
    )
    xb = (x * 0.0625).astype(ml_dtypes.bfloat16)  # fold the 1/16 avg scale
    # cols (d3, o4, e4) -> (e4, d3, o4)
    xb = np.ascontiguousarray(
        xb.reshape(32, 1024, 32, 16, 2).transpose(0, 1, 4, 2, 3)
    ).reshape(32, 1024, 1024)
    pm = _build_pm()
    if _NC_CACHE is None:
        _NC_CACHE = build_nc()
    nc = _NC_CACHE

    in_maps = []
    for i in range(N_CORES):
        xc = np.ascontiguousarray(
            xb[SLICES_PER_CORE * i : SLICES_PER_CORE * (i + 1)]
        ).reshape(ROWS, 1024)
        # block 6 rows (aL, e1, c2, e2) -> (aL, c2, e2, e1)
        blk = xc[3072:3584].reshape(8, 2, 16, 2, 1024)
        xc[3072:3584] = blk.transpose(0, 2, 3, 1, 4).reshape(512, 1024)
        in_maps.append({"x": xc, "pm": pm})

    res = run_bass_kernel_spmd(
        nc, in_maps, core_ids=list(range(N_CORES)), trace=_trace
    )
    # y3[q, 512j + f]: q = (o1l' 8, d2blk 8), f = (o2l 2, o3 16, o4 16);
    # block j -> (slice, khalf): o1 = 8*khalf + o1l', o2 = 2*d2blk + o2l.
    # y4[p, 16*o3 + o4]: slice 3, o1 = p//16 (in [0,8)), o2 = p%16.
    outs = []
    for i in range(N_CORES):
        arr = (
            res.results[i]["y3"]
            .astype(np.float32)
            .reshape(8, 8, 7, 2, 16, 16)
            .transpose(2, 0, 3, 1, 4, 5)  # [j, o1l', o2l, d2blk, o3, o4]
        )
        oc = np.empty((4, 16, 16, 16, 16), np.float32)
        for j, (s, kh) in enumerate(
            [(0, 0), (0, 1), (1, 0), (1, 1), (2, 0), (2, 1), (3, 1)]
        ):
            # note o2 = 2*d2blk + o2l -> need [o1l', d2blk, o2l] order
            oc[s, 8 * kh : 8 * kh + 8] = (
                arr[j].transpose(0, 2, 1, 3, 4).reshape(8, 16, 16, 16)
            )
        oc[3, 0:8] = (
            res.results[i]["y4"].astype(np.float32).reshape(8, 16, 16, 16)
        )
        outs.append(oc)
    out = np.concatenate(outs, axis=0).reshape(2, 16, 16, 16, 16, 16)
    out = np.ascontiguousarray(out).astype(np.float32)
    if _trace:
        kernel.last_results = res
    return out
